# revision 41
# baseline (speedup 1.0000x reference)
"""Trainium2 Bass kernel for Masked_Actor_Net_PNAConv (3x PNAConv + gated masked softmax head).

Sharding: data-parallel by graph across 8 NeuronCores (8 graphs / 2048 nodes /
16384 edges per core). Weights replicated. BatchNorm batch stats are
all-reduced across cores (one [128, 2*Fo] f32 AllReduce per conv layer).

Device-side structure (per core, per layer):
  - h kept feature-major in SBUF: hT [128, F, 2048] bf16
  - A = h @ Wm_src computed node-major on PE -> a_sb (node-major, [128n, 16t, cinp])
  - msgT gather done ON THE PE: host builds per-graph one-hot src-selection
    matrices S[nb][128 nodes, 2048 edge-slots]; msgT tile = A_blk^T @ S0 +
    A_blk^T @ S1 + Wmc^T @ es accumulated in PSUM (gather + edge term fused,
    exact: one-hot x bf16 is exact, msg rounds to bf16 once from f32)
  - DEG=8 aggregations: max/sumsq via DVE pairwise trees (edges j-interleaved
    per graph on host so tree operands are contiguous); squares on gpsimd;
    sum via PE matmul with host-built adjacency count matrices + es-sum term
  - h[dst] projection and all biases folded into U weights / BN on host
  - U matmuls run per graph-PAIR with N=512 moving tiles
  - BatchNorm folded into the mixing Linear: u centered with per-partition
    tensor_scalar sub, Wx rows scaled by gamma/sigma after the stats AllReduce
"""
import sys
sys.path.insert(0, '/opt/trn_rl_repo')
import contextlib
import numpy as np
import ml_dtypes

import concourse.bacc as bacc
import concourse.mybir as mybir
import concourse.bass_isa as bass_isa
from concourse import tile
from concourse.bass_utils import run_bass_kernel_spmd

BF = mybir.dt.bfloat16
F32 = mybir.dt.float32
U8 = mybir.dt.uint8
AL = mybir.AluOpType
AF = mybir.ActivationFunctionType
AX = mybir.AxisListType

B, NN, DEG = 64, 256, 8
N, E = B * NN, B * NN * DEG
IN_N, IN_E = 128, 16
TP = 192
H1 = 384
NCORES = 8
G = B // NCORES        # 8 graphs per core
NC = G * NN            # 2048 nodes per core
EC = NC * DEG          # 16384 edges per core

CIN = [IN_N, H1 + 32, H1]                 # 128, 416, 384
COUT = [H1, H1, TP]                       # 384, 384, 192
NF = [(c + 127) // 128 for c in CIN]      # 1, 4, 3
CINP = [128 * f for f in NF]              # 128, 512, 384
NFO = [(c + 127) // 128 for c in COUT]    # 3, 3, 2
CSZ = [[min(128, CIN[k] - 128 * i) for i in range(NF[k])] for k in range(3)]
MSZ = [[min(128, COUT[k] - 128 * i) for i in range(NFO[k])] for k in range(3)]

_BUILT = {}


def _bf(x):
    return np.ascontiguousarray(np.asarray(x, np.float32).astype(ml_dtypes.bfloat16))


def _f32(x):
    return np.ascontiguousarray(np.asarray(x, np.float32))


# ---------------------------------------------------------------------------
# device kernel (SPMD, identical program on all 8 cores)
# ---------------------------------------------------------------------------

def build_nc():
    import os as _os
    nc = bacc.Bacc(None, target_bir_lowering=False, debug=True)

    def par(name, shape, dt, out=False):
        return nc.declare_dram_parameter(name, list(shape), dt, isOutput=out)

    p_nsT = par("nsT", [128, 2048], BF)
    p_dmT = par("dmT", [128, 2 * 2048], BF)
    p_esT = par("esT", [16, EC], BF)
    p_sel = par("sel", [128, G * 2 * 2048], BF)
    p_mask = par("mask", [128, 16 * 192], BF)
    p_wma = [par(f"wma{k}", [128, NF[k] * CINP[k]], BF) for k in range(3)]
    p_wmc = [par(f"wmc{k}", [16, CINP[k]], BF) for k in range(3)]
    p_wu = [par(f"wu{k}", [128, 4 * NF[k] * COUT[k]], BF) for k in range(3)]
    p_wx = [par(f"wx{k}", [128, NFO[k] * COUT[k]], BF) for k in range(3)]
    p_gam = [par(f"gam{k}", [128, NFO[k]], F32) for k in range(3)]
    p_bh = [par(f"bh{k}", [128, NFO[k]], F32) for k in range(2)]
    p_w12 = par("w12", [128, 2 * 32], BF)
    p_b12 = par("b12", [32, 1], F32)
    p_w3 = par("w3", [128, 2 * 64], BF)
    p_b3 = par("b3", [64, 1], F32)
    p_w4 = par("w4", [64, 256], BF)
    p_b4 = par("b4", [128, 2], F32)
    p_out = par("out", [128, 16 * 192], F32, out=True)

    with tile.TileContext(nc) as tc:
        with contextlib.ExitStack() as ctx:
            stat = ctx.enter_context(tc.tile_pool(name="stat", bufs=1))
            selp = ctx.enter_context(tc.tile_pool(name="selp", bufs=2))
            espool = ctx.enter_context(tc.tile_pool(name="espool", bufs=1))
            cpool = ctx.enter_context(tc.tile_pool(name="cpool", bufs=3))   # msg slots
            tpool = ctx.enter_context(tc.tile_pool(name="tpool", bufs=1))   # tree scratch
            aggp = ctx.enter_context(tc.tile_pool(name="aggp", bufs=2))     # pair aggregates
            qpool = ctx.enter_context(tc.tile_pool(name="qpool", bufs=1))
            abuf = ctx.enter_context(tc.tile_pool(name="abuf", bufs=1))
            wupool = ctx.enter_context(tc.tile_pool(name="wupool", bufs=1))
            sml = ctx.enter_context(tc.tile_pool(name="sml", bufs=2))
            dpool = ctx.enter_context(tc.tile_pool(name="dpool", bufs=1, space="DRAM"))
            psW = ctx.enter_context(tc.tile_pool(name="psW", bufs=2, space="PSUM"))
            psC = ctx.enter_context(tc.tile_pool(name="psC", bufs=2, space="PSUM"))
            psU = ctx.enter_context(tc.tile_pool(name="psU", bufs=2, space="PSUM"))

            def load(shape, dt, src, tag, pool=None, eng=None):
                t = (pool or stat).tile(list(shape), dt, tag=tag, name=tag)
                (eng or nc.sync).dma_start(t[:], src[:])
                return t

            # startup-critical loads on the sync queue, in dependency order
            # (d2 needs w12+dmT; layer-0 A needs nsT+wma0); the rest drain on
            # the scalar engine's DMA queue in the background
            w12 = load([128, 2, 32], BF, p_w12, "w12")
            b12 = load([32, 1], F32, p_b12, "b12")
            dmT = cpool.tile([128, 2, 2048], BF, tag="csb", name="dmT")
            nc.sync.dma_start(dmT[:].rearrange("p c n -> p (c n)"), p_dmT[:])
            hT = [None, None, None]
            hT[0] = load([128, 1, 2048], BF, p_nsT, "hT0")
            hT[1] = stat.tile([128, 4, 2048], BF, tag="hT1", name="hT1")
            hT[2] = stat.tile([128, 3, 2048], BF, tag="hT2", name="hT2")
            uT = stat.tile([128, 3, 2048], BF, tag="uT")
            wma = [load([128, NF[k], CINP[k]], BF, p_wma[k], f"wma{k}s",
                        eng=(nc.sync if k == 0 else nc.scalar)) for k in range(3)]
            wmc = [load([16, CINP[k]], BF, p_wmc[k], f"wmc{k}s",
                        eng=(nc.sync if k == 0 else nc.scalar)) for k in range(3)]
            wx = [load([128, NFO[k], COUT[k]], BF, p_wx[k], f"wx{k}s", eng=nc.scalar) for k in range(3)]
            gam = [load([128, NFO[k]], F32, p_gam[k], f"gam{k}s", eng=nc.scalar) for k in range(3)]
            bh = [load([128, NFO[k]], F32, p_bh[k], f"bh{k}s", eng=nc.scalar) for k in range(2)]
            w3 = load([128, 2, 64], BF, p_w3, "w3", eng=nc.scalar)
            b3 = load([64, 1], F32, p_b3, "b3", eng=nc.scalar)
            w4 = load([64, 256], BF, p_w4, "w4", eng=nc.scalar)
            b4 = load([128, 2], F32, p_b4, "b4", eng=nc.scalar)
            wxs = stat.tile([128, 3, 384], BF, tag="wxs")
            cc_in = [dpool.tile([128, 2 * NFO[k]], F32, tag=f"ccin{k}", name=f"ccin{k}") for k in range(3)]
            cc_out = [dpool.tile([128, 2 * NFO[k]], F32, tag=f"ccout{k}", name=f"ccout{k}") for k in range(3)]
            cc_ind = dpool.tile([1, 2], F32, tag="ccind", name="ccind")
            cc_outd = dpool.tile([1, 2], F32, tag="ccoutd", name="ccoutd")

            # early dummy AllReduce: absorbs the cross-core launch skew under
            # layer-1 compute so the layer-1 BN AllReduce isn't the first sync
            import os as _os
            _rg = [[i] for i in range(NCORES)] if _os.environ.get("KERN_NO_CC") else [list(range(NCORES))]
            dmy = stat.tile([1, 2], F32, tag="dmy")
            nc.vector.memset(dmy[:], 0.0)
            nc.sync.dma_start(cc_ind[:], dmy[:])
            nc.gpsimd.collective_compute(
                "AllReduce", AL.add, replica_groups=_rg,
                ins=[cc_ind.opt()], outs=[cc_outd.opt()])

            # ---- d2 = dm @ (W1 @ W2) + b12 -> hT[1] chunk 3 rows 0:32 --------
            for n4 in range(4):
                ps = psW.tile([128, 512], F32, tag="psW")
                for kc in range(2):
                    nc.tensor.matmul(ps[0:32, :], w12[:, kc, :],
                                     dmT[:, kc, 512 * n4:512 * (n4 + 1)],
                                     start=(kc == 0), stop=(kc == 1))
                nc.scalar.activation(hT[1][0:32, 3, 512 * n4:512 * (n4 + 1)], ps[0:32, :],
                                     AF.Identity, bias=b12[:, 0:1])

            h3 = stat.tile([128, 16, 192], BF, tag="hT0")  # reuses hT0 slot (dead after layer 0)
            c30 = stat.tile([128, 1], F32, tag="c30")
            nc.vector.memset(c30[:], 1e-30)
            c5 = stat.tile([128, 1], F32, tag="c5")
            nc.vector.memset(c5[:], 1e-5)
            uaccS = stat.tile([128, 3, 4], F32, tag="uaccS")
            uaccQ = stat.tile([128, 3, 4], F32, tag="uaccQ")

            # ---- conv layers -------------------------------------------------
            PREF = 3   # graphs of layer k+1 whose C term is precomputed in the AR window
            cpre = []
            for k in range(3):
                F = NF[k]
                cinp, cout, Fo = CINP[k], COUT[k], NFO[k]
                csz, msz = CSZ[k], MSZ[k]
                h = hT[k]

                wu_k = load([128, 4 * F, cout], BF, p_wu[k], "wu_k", pool=wupool)
                nc.vector.memset(uaccS[:], 0.0)
                nc.vector.memset(uaccQ[:], 0.0)
                if k == 2:
                    nc.vector.memset(uT[64:128, 1, :], 0.0)

                # A = h @ Wma (node-major) -> a_sb
                a_sb = abuf.tile([128, 16, cinp], BF, tag="a_sb")
                for t in range(16):
                    ps = psW.tile([128, 512], F32, tag="psW")
                    for ki in range(F):
                        nc.tensor.matmul(ps[:, 0:cinp],
                                         h[0:csz[ki], ki, 128 * t:128 * (t + 1)],
                                         wma[k][0:csz[ki], ki, :],
                                         start=(ki == 0), stop=(ki == F - 1))
                    nc.scalar.activation(a_sb[:, t, :], ps[:, 0:cinp], AF.Copy, bias=0.0)

                # staged per-graph pipeline, 2 iterations of lag:
                #   A(it): gather+C matmuls (PE) + evictions (scalar only)
                #   B1(it-1): max + sum trees, mean (DVE)
                #   B2(it-2): square, sumsq tree, var, std (DVE + 1 scalar sqrt)
                #   U(p): after B2 of the pair's second graph
                csbs = {}
                pairs = {}

                def stage_A(g, fillers):
                    sel = selp.tile([128, 2, 2048], BF, tag="sel")
                    nc.sync.dma_start(sel[:].rearrange("p b n -> p (b n)"),
                                      p_sel[:, 4096 * g:4096 * (g + 1)])
                    pre = k > 0 and g < PREF
                    if pre:
                        csb = cpre[g]  # C term already evicted here at layer k-1 tail
                    else:
                        esg = espool.tile([16, 2048], BF, tag="esg")
                        nc.sync.dma_start(esg[:], p_esT[0:16, 2048 * g:2048 * (g + 1)])
                        csb = cpool.tile([128, 4, F, 512], BF, tag="csb")
                    csbf = csb[:].rearrange("p a f n -> p (a f n)")
                    for i2 in range(2 * F):
                        ps = psC.tile([128, 2, 512], F32, tag="psC")
                        for h2 in range(2):
                            flat = 2 * i2 + h2
                            e4, f = flat // F, flat % F
                            nc.tensor.matmul(ps[:, h2, :],
                                             a_sb[:, 2 * g, 128 * f:128 * (f + 1)],
                                             sel[:, 0, 512 * e4:512 * (e4 + 1)],
                                             start=True, stop=False)
                            nc.tensor.matmul(ps[:, h2, :],
                                             a_sb[:, 2 * g + 1, 128 * f:128 * (f + 1)],
                                             sel[:, 1, 512 * e4:512 * (e4 + 1)],
                                             start=False, stop=pre)
                            if not pre:
                                nc.tensor.matmul(ps[:, h2, :],
                                                 wmc[k][0:16, 128 * f:128 * (f + 1)],
                                                 esg[0:16, 512 * e4:512 * (e4 + 1)],
                                                 start=False, stop=True)
                        dst = csbf[:, 1024 * i2:1024 * (i2 + 1)]
                        psf = ps[:].rearrange("p h n -> p (h n)")
                        if pre:
                            nc.vector.tensor_tensor(dst, dst, psf, AL.add)
                        else:
                            nc.scalar.activation(dst, psf, AF.Copy, bias=0.0)
                        if fillers:
                            fillers.pop(0)()
                    csbs[g] = csb

                def stage_B1(g):
                    p, gg = g // 2, g % 2
                    if gg == 0:
                        pairs[p] = (aggp.tile([128, F, 2, 256], BF, tag="pme", name="pme"),
                                    aggp.tile([128, F, 2, 256], BF, tag="pmx", name="pmx"),
                                    aggp.tile([128, F, 2, 256], BF, tag="pst", name="pst"),
                                    sml.tile([128, 2, F, 256], BF, tag="pm2", name="pm2"))
                    pme, pmx, pst, pm2 = pairs[p]
                    csb = csbs[g]
                    # layout [p, e(4), f, 512]: edge j = 2e + (i // 256), node n = i % 256
                    tscr = tpool.tile([128, 2, F, 512], BF, tag="tscr")
                    nc.vector.tensor_tensor(tscr[:], csb[:, 0:2, :, :],
                                            csb[:, 2:4, :, :], AL.max)
                    nc.vector.tensor_tensor(tscr[:, 0, :, :], tscr[:, 0, :, :],
                                            tscr[:, 1, :, :], AL.max)
                    nc.vector.tensor_tensor(pmx[:, :, gg, :], tscr[:, 0, :, 0:256],
                                            tscr[:, 0, :, 256:512], AL.max)
                    nc.vector.tensor_tensor(tscr[:], csb[:, 0:2, :, :],
                                            csb[:, 2:4, :, :], AL.add)
                    nc.vector.tensor_tensor(tscr[:, 0, :, :], tscr[:, 0, :, :],
                                            tscr[:, 1, :, :], AL.add)
                    ssum = qpool.tile([128, F, 256], BF, tag="ssum")
                    nc.vector.tensor_tensor(ssum[:], tscr[:, 0, :, 0:256],
                                            tscr[:, 0, :, 256:512], AL.add)
                    nc.vector.tensor_scalar(pme[:, :, gg, :], ssum[:], 0.125, None, AL.mult)
                    nc.vector.tensor_tensor(pm2[:, gg, :, :], pme[:, :, gg, :],
                                            pme[:, :, gg, :], AL.mult)

                def sq_fillers(g):
                    # square csb(g) in place, chunked by e4: two chunks emitted
                    # on the scalar queue between evictions (fillers), two on DVE
                    csb = csbs[g]
                    return [lambda e4=e4: nc.scalar.activation(
                        csb[:, e4, :, :], csb[:, e4, :, :], AF.Square) for e4 in (0, 1)]

                def stage_B2(g):
                    p, gg = g // 2, g % 2
                    pme, pmx, pst, pm2 = pairs[p]
                    csb = csbs.pop(g)
                    for e4 in (2, 3):
                        nc.vector.tensor_tensor(csb[:, e4, :, :], csb[:, e4, :, :],
                                                csb[:, e4, :, :], AL.mult)
                    tscr = tpool.tile([128, 2, F, 512], BF, tag="tscr")
                    nc.vector.tensor_tensor(tscr[:], csb[:, 0:2, :, :],
                                            csb[:, 2:4, :, :], AL.add)
                    nc.vector.tensor_tensor(tscr[:, 0, :, :], tscr[:, 0, :, :],
                                            tscr[:, 1, :, :], AL.add)
                    qsum = qpool.tile([128, F, 256], F32, tag="qsum")
                    nc.vector.tensor_tensor(qsum[:], tscr[:, 0, :, 0:256],
                                            tscr[:, 0, :, 256:512], AL.add)
                    # var = relu(qsum/8 - mean^2), std = sqrt(var + 1e-30)
                    nc.vector.tensor_scalar(qsum[:], qsum[:], 0.125, None, AL.mult)
                    nc.vector.tensor_tensor(qsum[:], qsum[:], pm2[:, gg, :, :], AL.subtract)
                    nc.vector.tensor_scalar(qsum[:], qsum[:], 0.0, None, AL.max)
                    nc.scalar.activation(pst[:, :, gg, :], qsum[:], AF.Sqrt, bias=c30[:, 0:1])

                def stage_U(p):
                    pme, pmx, pst, pm2 = pairs.pop(p)
                    xs = [None, pme, pmx, pst]
                    for mo in range(Fo):
                        mi = msz[mo]
                        ps = psU.tile([128, 512], F32, tag="psU")
                        nmm = 4 * F
                        i = 0
                        for sect in range(4):
                            for f in range(F):
                                if sect == 0:
                                    rhs = h[0:csz[f], f, 512 * p:512 * (p + 1)]
                                else:
                                    rhs = xs[sect][0:csz[f], f, :, :]
                                nc.tensor.matmul(
                                    ps[0:mi, :],
                                    wu_k[0:csz[f], sect * F + f, 128 * mo:128 * mo + mi],
                                    rhs, start=(i == 0), stop=(i == nmm - 1))
                                i += 1
                        nc.scalar.activation(uT[0:mi, mo, 512 * p:512 * (p + 1)], ps[0:mi, :],
                                             AF.Copy, bias=0.0,
                                             accum_out=uaccS[0:mi, mo, p:p + 1])
                        usq = sml.tile([128, 512], BF, tag="usq")
                        nc.scalar.activation(usq[0:mi, :], uT[0:mi, mo, 512 * p:512 * (p + 1)],
                                             AF.Square, accum_out=uaccQ[0:mi, mo, p:p + 1])

                for it in range(G + 3):
                    fillers = sq_fillers(it - 2) if 2 <= it <= G + 1 else []
                    if it < G:
                        stage_A(it, fillers)
                    for fop in fillers:
                        fop()
                    if 1 <= it <= G:
                        stage_B1(it - 1)
                    if 2 <= it <= G + 1:
                        stage_B2(it - 2)
                    if it >= 3 and (it - 3) % 2 == 1:
                        stage_U((it - 3) // 2)

                # ---- prefetch next layer's C = es @ Wmc into the AR window ----
                cpre = []
                if k < 2:
                    F1 = NF[k + 1]
                    for gp in range(PREF):
                        esg = espool.tile([16, 2048], BF, tag="esg")
                        nc.sync.dma_start(esg[:], p_esT[0:16, 2048 * gp:2048 * (gp + 1)])
                        cp = cpool.tile([128, 4, F1, 512], BF, tag="csb")
                        cpf = cp[:].rearrange("p a f n -> p (a f n)")
                        for i2 in range(2 * F1):
                            ps = psC.tile([128, 2, 512], F32, tag="psC")
                            for h2 in range(2):
                                flat = 2 * i2 + h2
                                e4, f = flat // F1, flat % F1
                                nc.tensor.matmul(ps[:, h2, :],
                                                 wmc[k + 1][0:16, 128 * f:128 * (f + 1)],
                                                 esg[0:16, 512 * e4:512 * (e4 + 1)],
                                                 start=True, stop=True)
                            dst = cpf[:, 1024 * i2:1024 * (i2 + 1)]
                            psf = ps[:].rearrange("p h n -> p (h n)")
                            if i2 % 2 == 0:
                                nc.scalar.activation(dst, psf, AF.Copy, bias=0.0)
                            else:
                                nc.vector.tensor_copy(dst, psf)
                        cpre.append(cp)

                # ---- BN stats all-reduce, fold into mixing ----
                # chunk-reduce on the scalar queue (fires right after the last
                # U evict; the vector queue still has a tree backlog here)
                ccs = stat.tile([128, 6], F32, tag="ccs")
                racc = sml.tile([128, 4], BF, tag="usq", name="racc")
                for mo in range(Fo):
                    nc.scalar.activation(racc[:, :], uaccS[:, mo, :], AF.Copy, bias=0.0,
                                         accum_out=ccs[:, mo:mo + 1])
                    nc.scalar.activation(racc[:, :], uaccQ[:, mo, :], AF.Copy, bias=0.0,
                                         accum_out=ccs[:, Fo + mo:Fo + mo + 1])
                nc.sync.dma_start(cc_in[k][:], ccs[:, 0:2 * Fo])
                nc.gpsimd.collective_compute(
                    "AllReduce", AL.add, replica_groups=_rg,
                    ins=[cc_in[k].opt()], outs=[cc_out[k].opt()])
                ccr = stat.tile([128, 6], F32, tag="ccr")
                nc.sync.dma_start(ccr[:, 0:2 * Fo], cc_out[k][:])
                mu = stat.tile([128, 3], F32, tag="mu")
                sc = stat.tile([128, 3], F32, tag="sc")
                mu2 = stat.tile([128, 3], F32, tag="mu2")
                nc.scalar.activation(mu[:, 0:Fo], ccr[:, 0:Fo], AF.Copy, bias=0.0, scale=1.0 / N)
                nc.scalar.activation(sc[:, 0:Fo], ccr[:, Fo:2 * Fo], AF.Copy, bias=0.0, scale=1.0 / N)
                nc.vector.tensor_tensor(mu2[:, 0:Fo], mu[:, 0:Fo], mu[:, 0:Fo], AL.mult)
                nc.vector.tensor_tensor(sc[:, 0:Fo], sc[:, 0:Fo], mu2[:, 0:Fo], AL.subtract)
                nc.scalar.activation(sc[:, 0:Fo], sc[:, 0:Fo], AF.Sqrt, bias=c5[:, 0:1])
                nc.vector.reciprocal(sc[:, 0:Fo], sc[:, 0:Fo])
                nc.vector.tensor_tensor(sc[:, 0:Fo], sc[:, 0:Fo], gam[k][:, 0:Fo], AL.mult)
                for mo in range(Fo):
                    mi = msz[mo]
                    nc.vector.tensor_scalar(uT[0:mi, mo, :], uT[0:mi, mo, :],
                                            mu[0:mi, mo:mo + 1], None, AL.subtract)
                    nc.vector.tensor_scalar(wxs[:, mo, 0:cout], wx[k][:, mo, 0:cout],
                                            sc[:, mo:mo + 1], None, AL.mult)
                if k == 2:
                    nc.vector.memset(uT[64:65, 1, :], 1.0)
                # mixing matmul (+ BN shift via bias / ones-row), relu(leaky) = relu
                if k < 2:
                    hn = hT[k + 1]
                    for mo in range(Fo):
                        for n4 in range(4):
                            ps = psW.tile([128, 512], F32, tag="psW")
                            for mk in range(Fo):
                                nc.tensor.matmul(ps[:, :],
                                                 wxs[0:msz[mk], mk, 128 * mo:128 * (mo + 1)],
                                                 uT[0:msz[mk], mk, 512 * n4:512 * (n4 + 1)],
                                                 start=(mk == 0), stop=(mk == Fo - 1))
                            nc.scalar.activation(hn[:, mo, 512 * n4:512 * (n4 + 1)], ps[:, :],
                                                 AF.Relu, bias=bh[k][:, mo:mo + 1])
                else:
                    nmx = stat.tile([128, 16], BF, tag="nmx")
                    for t in range(16):
                        ps = psW.tile([128, 512], F32, tag="psW")
                        nc.tensor.matmul(ps[:, 0:192], uT[0:128, 0, 128 * t:128 * (t + 1)],
                                         wxs[0:128, 0, 0:192], start=True, stop=False)
                        nc.tensor.matmul(ps[:, 0:192], uT[0:65, 1, 128 * t:128 * (t + 1)],
                                         wxs[0:65, 1, 0:192], start=False, stop=True)
                        nc.scalar.activation(h3[:, t, :], ps[:, 0:192], AF.Lrelu, alpha=0.01)
                        nc.vector.tensor_reduce(nmx[:, t:t + 1], h3[:, t:t + 1, :], AX.X, AL.max)

            # ---- head --------------------------------------------------------
            ps3 = psW.tile([128, 512], F32, tag="psW")
            nc.tensor.matmul(ps3[0:64, 0:8], w3[:, 0, :], nmx[:, 0::2], start=True, stop=False)
            nc.tensor.matmul(ps3[0:64, 0:8], w3[:, 1, :], nmx[:, 1::2], start=False, stop=True)
            r3 = stat.tile([64, 8], BF, tag="r3")
            nc.scalar.activation(r3[:], ps3[0:64, 0:8], AF.Relu, bias=b3[:, 0:1])
            gn = stat.tile([128, 16], F32, tag="gn")
            for half in range(2):
                ps4 = psW.tile([128, 512], F32, tag="psW")
                nc.tensor.matmul(ps4[:, 0:8], w4[0:64, 128 * half:128 * (half + 1)], r3[:],
                                 start=True, stop=True)
                nc.scalar.activation(gn[:, half::2], ps4[:, 0:8], AF.Sigmoid,
                                     bias=b4[:, half:half + 1])
            mask = cpool.tile([128, 16, 192], BF, tag="csb", name="maskt")
            nc.scalar.dma_start(mask[:], p_mask[:])
            feat = cpool.tile([128, 16, 192], F32, tag="csb")
            for c in range(16):
                nc.vector.tensor_scalar(feat[:, c, :], h3[:, c, :], gn[:, c:c + 1], None, AL.mult)
            # softmax shifted by the unmasked max (shift-invariant); mask after exp
            gmax = stat.tile([128, 8], F32, tag="gmax")
            gmaxr = stat.tile([128, 8], F32, tag="gmaxr")
            nc.vector.tensor_reduce(gmax[:], feat[:].rearrange("p (g x) t -> p g (x t)", g=8), AX.X, AL.max)
            nc.gpsimd.partition_all_reduce(gmaxr[:], gmax[:], 128, bass_isa.ReduceOp.max)
            nc.vector.tensor_scalar(gmaxr[:], gmaxr[:], -1.0, None, AL.mult)
            for g in range(8):
                nc.scalar.activation(feat[:, 2 * g:2 * (g + 1), :], feat[:, 2 * g:2 * (g + 1), :],
                                     AF.Exp, bias=gmaxr[:, g:g + 1])
            nc.vector.tensor_tensor(feat[:], feat[:], mask[:], AL.mult)
            gsum = stat.tile([128, 8], F32, tag="gsum")
            gsumr = stat.tile([128, 8], F32, tag="gsumr")
            nc.vector.tensor_reduce(gsum[:], feat[:].rearrange("p (g x) t -> p g (x t)", g=8), AX.X, AL.add)
            nc.gpsimd.partition_all_reduce(gsumr[:], gsum[:], 128, bass_isa.ReduceOp.add)
            nc.vector.reciprocal(gsumr[:], gsumr[:])
            osb = cpool.tile([128, 16, 192], F32, tag="csb")
            for g in range(8):
                nc.vector.tensor_scalar(osb[:, 2 * g:2 * (g + 1), :], feat[:, 2 * g:2 * (g + 1), :],
                                        gsumr[:, g:g + 1], None, AL.mult)
            nc.sync.dma_start(p_out[:], osb[:].rearrange("p c t -> p (c t)"))

    nc.compile()
    return nc


# ---------------------------------------------------------------------------
# host prep + launch
# ---------------------------------------------------------------------------

def prepare_in_maps(inputs):
    src = np.asarray(inputs["src"], np.int64)
    dst = np.asarray(inputs["dst"], np.int64)
    assert np.array_equal(dst, np.repeat(np.arange(N, dtype=np.int64), DEG)), "dst structure"
    assert np.array_equal(src // NN, dst // NN), "edges must be graph-local"

    ns = _f32(inputs["ns"]); es = _f32(inputs["es"]); dm = _f32(inputs["dm"])
    mask_fv = _f32(inputs["mask_fv"])

    Wm = [_f32(inputs[f"Wm{k + 1}"]) for k in range(3)]
    Wu = [_f32(inputs[f"Wu{k + 1}"]) for k in range(3)]
    Wx = [_f32(inputs[f"Wx{k + 1}"]) for k in range(3)]
    bx = [_f32(inputs[f"bx{k + 1}"]) for k in range(3)]
    bng = [_f32(inputs[f"bng{k + 1}"]) for k in range(3)]
    bnb = [_f32(inputs[f"bnb{k + 1}"]) for k in range(3)]

    wma_u, wmc_u, wu_u, wx_u, gam_u, bh_u = [], [], [], [], [], []
    for k in range(3):
        cin, cout, Fk, cinp, Fo = CIN[k], COUT[k], NF[k], CINP[k], NFO[k]
        Wma, Wmb, Wmce = Wm[k][:cin], Wm[k][cin:2 * cin], Wm[k][2 * cin:]
        Wmean = Wu[k][cin:2 * cin] + 8.0 * Wu[k][3 * cin:4 * cin]
        Wmax = Wu[k][2 * cin:3 * cin]
        Wstd = Wu[k][4 * cin:]
        Wh = Wu[k][:cin] + Wmb @ (Wmean + Wmax)
        a = np.zeros((128, Fk, cinp), np.float32)
        for ki in range(Fk):
            a[0:CSZ[k][ki], ki, :cin] = Wma[128 * ki:128 * ki + CSZ[k][ki]]
        wma_u.append(_bf(a.reshape(128, -1)))
        c = np.zeros((16, cinp), np.float32)
        c[:, :cin] = Wmce
        wmc_u.append(_bf(c))
        u = np.zeros((128, 4 * Fk, cout), np.float32)
        for si, Wsec in enumerate([Wh, Wmean, Wmax, Wstd]):
            for f in range(Fk):
                u[0:CSZ[k][f], si * Fk + f, :] = Wsec[128 * f:128 * f + CSZ[k][f]]
        wu_u.append(_bf(u.reshape(128, -1)))
        if k < 2:
            x = np.zeros((128, Fo, cout), np.float32)
            gcol = np.zeros((128, Fo), np.float32)
            bcol = np.zeros((128, Fo), np.float32)
            bhv = bnb[k] @ Wx[k] + bx[k]
            for mk in range(Fo):
                m = MSZ[k][mk]
                x[0:m, mk, :] = Wx[k][128 * mk:128 * mk + m]
                gcol[0:m, mk] = bng[k][128 * mk:128 * mk + m]
                bcol[0:m, mk] = bhv[128 * mk:128 * mk + m]
            wx_u.append(_bf(x.reshape(128, -1)))
            gam_u.append(_f32(gcol))
            bh_u.append(_f32(bcol))
        else:
            x = np.zeros((128, 2, cout), np.float32)
            x[0:128, 0, :] = Wx[k][0:128]
            x[0:64, 1, :] = Wx[k][128:192]
            x[64, 1, :] = bnb[k] @ Wx[k] + bx[k]       # bias row (pairs with u ones-row)
            wx_u.append(_bf(x.reshape(128, -1)))
            gcol = np.zeros((128, 2), np.float32)
            gcol[0:128, 0] = bng[k][0:128]
            gcol[0:64, 1] = bng[k][128:192]
            gcol[64, 1] = np.sqrt(np.float32(1e-5))    # scale row becomes exactly 1.0
            gam_u.append(_f32(gcol))

    W12 = _f32(inputs["W1"]) @ _f32(inputs["W2"])
    b12v = _f32(inputs["b1"]) @ _f32(inputs["W2"]) + _f32(inputs["b2"])
    w12_u = _bf(W12.reshape(2, 128, 32).transpose(1, 0, 2).reshape(128, -1))
    w3_u = _bf(_f32(inputs["W3"]).reshape(2, 128, 64).transpose(1, 0, 2).reshape(128, -1))
    w4_u = _bf(inputs["W4"])
    b4_u = _f32(np.asarray(inputs["b4"]).reshape(2, 128).T)

    shared = {
        **{f"wma{k}": wma_u[k] for k in range(3)},
        **{f"wmc{k}": wmc_u[k] for k in range(3)},
        **{f"wu{k}": wu_u[k] for k in range(3)},
        **{f"wx{k}": wx_u[k] for k in range(3)},
        **{f"gam{k}": gam_u[k] for k in range(3)},
        **{f"bh{k}": bh_u[k] for k in range(2)},
        "w12": w12_u, "b12": _f32(b12v.reshape(32, 1)),
        "w3": w3_u, "b3": _f32(np.asarray(inputs["b3"]).reshape(64, 1)),
        "w4": w4_u, "b4": b4_u,
    }

    in_maps = []
    ecols = np.arange(2048)
    for c in range(NCORES):
        n0 = NC * c
        gg, jj, nn2 = np.meshgrid(np.arange(G), np.arange(DEG), np.arange(NN), indexing="ij")
        perm = (8 * (n0 + 256 * gg + nn2) + jj).reshape(-1)
        esl = es[perm]
        # per-graph one-hot src selectors: sel[row, g, nb, j*256+n] = 1
        # iff src of edge (g, j, n) is node nb*128 + row of graph g
        srcg = (src[perm] - n0).reshape(G, DEG * NN).astype(np.int64)
        srcg -= 256 * (np.arange(G)[:, None])            # graph-local 0..255
        sel = np.zeros((128, G, 2, 2048), ml_dtypes.bfloat16)
        gidx = np.repeat(np.arange(G), 2048)
        sel[(srcg % 128).reshape(-1), gidx, (srcg // 128).reshape(-1), np.tile(ecols, G)] = 1.0
        in_maps.append({
            "nsT": _bf(ns[n0:n0 + NC].T),
            "dmT": _bf(dm[n0:n0 + NC].T.reshape(2, 128, 2048).transpose(1, 0, 2).reshape(128, -1)),
            "esT": _bf(esl.T),
            "sel": np.ascontiguousarray(sel.reshape(128, -1)),
            "mask": _bf(mask_fv[n0:n0 + NC].reshape(16, 128, 192).transpose(1, 0, 2)
                        .reshape(128, -1)),
            **shared,
        })

    return in_maps


def collect_out(res):
    out = np.zeros((B, NN * TP), np.float32)
    for c in range(NCORES):
        oc = res.results[c]["out"].reshape(128, 16, 192).transpose(1, 0, 2).reshape(NC, TP)
        out[G * c:G * (c + 1)] = oc.reshape(G, NN * TP)
    return out


def kernel(**inputs):
    in_maps = prepare_in_maps(inputs)
    nc = _BUILT.get("nc")
    if nc is None:
        nc = build_nc()
        _BUILT["nc"] = nc
    res = run_bass_kernel_spmd(nc, in_maps, list(range(NCORES)))
    _BUILT["last_results"] = res
    return collect_out(res)


# revision 42
# speedup vs baseline: 1.0298x; 1.0298x over previous
"""Trainium2 Bass kernel for Masked_Actor_Net_PNAConv (3x PNAConv + gated masked softmax head).

Sharding: data-parallel by graph across 8 NeuronCores (8 graphs / 2048 nodes /
16384 edges per core). Weights replicated. BatchNorm batch stats are
all-reduced across cores (one [128, 2*Fo] f32 AllReduce per conv layer).

Device-side structure (per core, per layer):
  - h kept feature-major in SBUF: hT [128, F, 2048] bf16
  - A = h @ Wm_src computed node-major on PE -> a_sb (node-major, [128n, 16t, cinp])
  - msgT gather done ON THE PE: host builds per-graph one-hot src-selection
    matrices S[nb][128 nodes, 2048 edge-slots]; msgT tile = A_blk^T @ S0 +
    A_blk^T @ S1 + Wmc^T @ es accumulated in PSUM (gather + edge term fused,
    exact: one-hot x bf16 is exact, msg rounds to bf16 once from f32)
  - DEG=8 aggregations: max/sumsq via DVE pairwise trees (edges j-interleaved
    per graph on host so tree operands are contiguous); squares on gpsimd;
    sum via PE matmul with host-built adjacency count matrices + es-sum term
  - h[dst] projection and all biases folded into U weights / BN on host
  - U matmuls run per graph-PAIR with N=512 moving tiles
  - BatchNorm folded into the mixing Linear: u centered with per-partition
    tensor_scalar sub, Wx rows scaled by gamma/sigma after the stats AllReduce
"""
import sys
sys.path.insert(0, '/opt/trn_rl_repo')
import contextlib
import numpy as np
import ml_dtypes

import concourse.bacc as bacc
import concourse.mybir as mybir
import concourse.bass_isa as bass_isa
from concourse import tile
from concourse.bass_utils import run_bass_kernel_spmd

BF = mybir.dt.bfloat16
F32 = mybir.dt.float32
U8 = mybir.dt.uint8
AL = mybir.AluOpType
AF = mybir.ActivationFunctionType
AX = mybir.AxisListType

B, NN, DEG = 64, 256, 8
N, E = B * NN, B * NN * DEG
IN_N, IN_E = 128, 16
TP = 192
H1 = 384
NCORES = 8
G = B // NCORES        # 8 graphs per core
NC = G * NN            # 2048 nodes per core
EC = NC * DEG          # 16384 edges per core

CIN = [IN_N, H1 + 32, H1]                 # 128, 416, 384
COUT = [H1, H1, TP]                       # 384, 384, 192
NF = [(c + 127) // 128 for c in CIN]      # 1, 4, 3
CINP = [128 * f for f in NF]              # 128, 512, 384
NFO = [(c + 127) // 128 for c in COUT]    # 3, 3, 2
CSZ = [[min(128, CIN[k] - 128 * i) for i in range(NF[k])] for k in range(3)]
MSZ = [[min(128, COUT[k] - 128 * i) for i in range(NFO[k])] for k in range(3)]

_BUILT = {}


def _bf(x):
    return np.ascontiguousarray(np.asarray(x, np.float32).astype(ml_dtypes.bfloat16))


def _f32(x):
    return np.ascontiguousarray(np.asarray(x, np.float32))


# ---------------------------------------------------------------------------
# device kernel (SPMD, identical program on all 8 cores)
# ---------------------------------------------------------------------------

def build_nc():
    import os as _os
    nc = bacc.Bacc(None, target_bir_lowering=False, debug=True)

    def par(name, shape, dt, out=False):
        return nc.declare_dram_parameter(name, list(shape), dt, isOutput=out)

    p_nsT = par("nsT", [128, 2048], BF)
    p_dmT = par("dmT", [128, 2 * 2048], BF)
    p_esT = par("esT", [16, EC], BF)
    p_sel = par("sel", [128, G * 2 * 2048], BF)
    p_mask = par("mask", [128, 16 * 192], BF)
    p_wma = [par(f"wma{k}", [128, NF[k] * CINP[k]], BF) for k in range(3)]
    p_wmc = [par(f"wmc{k}", [16, CINP[k]], BF) for k in range(3)]
    p_wu = [par(f"wu{k}", [128, 4 * NF[k] * COUT[k]], BF) for k in range(3)]
    p_wx = [par(f"wx{k}", [128, NFO[k] * COUT[k]], BF) for k in range(3)]
    p_gam = [par(f"gam{k}", [128, NFO[k]], F32) for k in range(3)]
    p_bh = [par(f"bh{k}", [128, NFO[k]], F32) for k in range(2)]
    p_w12 = par("w12", [128, 2 * 32], BF)
    p_b12 = par("b12", [32, 1], F32)
    p_w3 = par("w3", [128, 2 * 64], BF)
    p_b3 = par("b3", [64, 1], F32)
    p_w4 = par("w4", [64, 256], BF)
    p_b4 = par("b4", [128, 2], F32)
    p_out = par("out", [128, 16 * 192], F32, out=True)

    with tile.TileContext(nc) as tc:
        with contextlib.ExitStack() as ctx:
            stat = ctx.enter_context(tc.tile_pool(name="stat", bufs=1))
            selp = ctx.enter_context(tc.tile_pool(name="selp", bufs=2))
            espool = ctx.enter_context(tc.tile_pool(name="espool", bufs=1))
            cpool = ctx.enter_context(tc.tile_pool(name="cpool", bufs=3))   # msg slots
            tpool = ctx.enter_context(tc.tile_pool(name="tpool", bufs=1))   # tree scratch
            aggp = ctx.enter_context(tc.tile_pool(name="aggp", bufs=2))     # pair aggregates
            qpool = ctx.enter_context(tc.tile_pool(name="qpool", bufs=1))
            abuf = ctx.enter_context(tc.tile_pool(name="abuf", bufs=1))
            wupool = ctx.enter_context(tc.tile_pool(name="wupool", bufs=1))
            sml = ctx.enter_context(tc.tile_pool(name="sml", bufs=2))
            dpool = ctx.enter_context(tc.tile_pool(name="dpool", bufs=1, space="DRAM"))
            psW = ctx.enter_context(tc.tile_pool(name="psW", bufs=2, space="PSUM"))
            psC = ctx.enter_context(tc.tile_pool(name="psC", bufs=2, space="PSUM"))
            psU = ctx.enter_context(tc.tile_pool(name="psU", bufs=2, space="PSUM"))

            def load(shape, dt, src, tag, pool=None, eng=None):
                t = (pool or stat).tile(list(shape), dt, tag=tag, name=tag)
                (eng or nc.sync).dma_start(t[:], src[:])
                return t

            # startup-critical loads on the sync queue, in dependency order
            # (d2 needs w12+dmT; layer-0 A needs nsT+wma0); the rest drain on
            # the scalar engine's DMA queue in the background
            w12 = load([128, 2, 32], BF, p_w12, "w12")
            b12 = load([32, 1], F32, p_b12, "b12")
            dmT = cpool.tile([128, 2, 2048], BF, tag="csb", name="dmT")
            nc.sync.dma_start(dmT[:].rearrange("p c n -> p (c n)"), p_dmT[:])
            hT = [None, None, None]
            hT[0] = load([128, 1, 2048], BF, p_nsT, "hT0")
            hT[1] = stat.tile([128, 4, 2048], BF, tag="hT1", name="hT1")
            hT[2] = stat.tile([128, 3, 2048], BF, tag="hT2", name="hT2")
            uT = stat.tile([128, 3, 2048], BF, tag="uT")
            wma = [load([128, NF[k], CINP[k]], BF, p_wma[k], f"wma{k}s",
                        eng=(nc.sync if k == 0 else nc.scalar)) for k in range(3)]
            wmc = [load([16, CINP[k]], BF, p_wmc[k], f"wmc{k}s",
                        eng=(nc.sync if k == 0 else nc.scalar)) for k in range(3)]
            wx = [load([128, NFO[k], COUT[k]], BF, p_wx[k], f"wx{k}s", eng=nc.scalar) for k in range(3)]
            gam = [load([128, NFO[k]], F32, p_gam[k], f"gam{k}s", eng=nc.scalar) for k in range(3)]
            bh = [load([128, NFO[k]], F32, p_bh[k], f"bh{k}s", eng=nc.scalar) for k in range(2)]
            w3 = load([128, 2, 64], BF, p_w3, "w3", eng=nc.scalar)
            b3 = load([64, 1], F32, p_b3, "b3", eng=nc.scalar)
            w4 = load([64, 256], BF, p_w4, "w4", eng=nc.scalar)
            b4 = load([128, 2], F32, p_b4, "b4", eng=nc.scalar)
            wxs = stat.tile([128, 3, 384], BF, tag="wxs")
            cc_in = [dpool.tile([128, 2 * NFO[k]], F32, tag=f"ccin{k}", name=f"ccin{k}") for k in range(3)]
            cc_out = [dpool.tile([128, 2 * NFO[k]], F32, tag=f"ccout{k}", name=f"ccout{k}") for k in range(3)]
            cc_ind = dpool.tile([1, 2], F32, tag="ccind", name="ccind")
            cc_outd = dpool.tile([1, 2], F32, tag="ccoutd", name="ccoutd")

            # early dummy AllReduce: absorbs the cross-core launch skew under
            # layer-1 compute so the layer-1 BN AllReduce isn't the first sync
            import os as _os
            _rg = [[i] for i in range(NCORES)] if _os.environ.get("KERN_NO_CC") else [list(range(NCORES))]
            dmy = stat.tile([1, 2], F32, tag="dmy")
            nc.vector.memset(dmy[:], 0.0)
            nc.sync.dma_start(cc_ind[:], dmy[:])
            nc.gpsimd.collective_compute(
                "AllReduce", AL.add, replica_groups=_rg,
                ins=[cc_ind.opt()], outs=[cc_outd.opt()])

            # ---- d2 = dm @ (W1 @ W2) + b12 -> hT[1] chunk 3 rows 0:32 --------
            for n4 in range(4):
                ps = psW.tile([128, 512], F32, tag="psW")
                for kc in range(2):
                    nc.tensor.matmul(ps[0:32, :], w12[:, kc, :],
                                     dmT[:, kc, 512 * n4:512 * (n4 + 1)],
                                     start=(kc == 0), stop=(kc == 1))
                nc.scalar.activation(hT[1][0:32, 3, 512 * n4:512 * (n4 + 1)], ps[0:32, :],
                                     AF.Identity, bias=b12[:, 0:1])

            h3 = stat.tile([128, 16, 192], BF, tag="hT0")  # reuses hT0 slot (dead after layer 0)
            c30 = stat.tile([128, 1], F32, tag="c30")
            nc.vector.memset(c30[:], 1e-30)
            c5 = stat.tile([128, 1], F32, tag="c5")
            nc.vector.memset(c5[:], 1e-5)
            uaccS = stat.tile([128, 3, 4], F32, tag="uaccS")
            uaccQ = stat.tile([128, 3, 4], F32, tag="uaccQ")

            # ---- conv layers -------------------------------------------------
            PREF = 3   # graphs of layer k+1 whose C term is precomputed in the AR window
            cpre = []
            for k in range(3):
                F = NF[k]
                cinp, cout, Fo = CINP[k], COUT[k], NFO[k]
                csz, msz = CSZ[k], MSZ[k]
                h = hT[k]

                wu_k = load([128, 4 * F, cout], BF, p_wu[k], "wu_k", pool=wupool)
                nc.vector.memset(uaccS[:], 0.0)
                nc.vector.memset(uaccQ[:], 0.0)
                if k == 2:
                    nc.vector.memset(uT[64:128, 1, :], 0.0)

                # A = h @ Wma (node-major) -> a_sb
                a_sb = abuf.tile([128, 16, cinp], BF, tag="a_sb")
                for t in range(16):
                    ps = psW.tile([128, 512], F32, tag="psW")
                    for ki in range(F):
                        nc.tensor.matmul(ps[:, 0:cinp],
                                         h[0:csz[ki], ki, 128 * t:128 * (t + 1)],
                                         wma[k][0:csz[ki], ki, :],
                                         start=(ki == 0), stop=(ki == F - 1))
                    nc.scalar.activation(a_sb[:, t, :], ps[:, 0:cinp], AF.Copy, bias=0.0)

                # staged per-graph pipeline, 2 iterations of lag:
                #   A(it): gather+C matmuls (PE) + evictions (scalar only)
                #   B1(it-1): max + sum trees, mean (DVE)
                #   B2(it-2): square, sumsq tree, var, std (DVE + 1 scalar sqrt)
                #   U(p): after B2 of the pair's second graph
                csbs = {}
                pairs = {}

                def stage_A(g, fillers):
                    sel = selp.tile([128, 2, 2048], BF, tag="sel")
                    nc.sync.dma_start(sel[:].rearrange("p b n -> p (b n)"),
                                      p_sel[:, 4096 * g:4096 * (g + 1)])
                    pre = k > 0 and g < PREF
                    if pre:
                        csb = cpre[g]  # C term already evicted here at layer k-1 tail
                    else:
                        esg = espool.tile([16, 2048], BF, tag="esg")
                        nc.sync.dma_start(esg[:], p_esT[0:16, 2048 * g:2048 * (g + 1)])
                        csb = cpool.tile([128, 4, F, 512], BF, tag="csb")
                    csbf = csb[:].rearrange("p a f n -> p (a f n)")
                    for i2 in range(2 * F):
                        ps = psC.tile([128, 2, 512], F32, tag="psC")
                        for h2 in range(2):
                            flat = 2 * i2 + h2
                            e4, f = flat // F, flat % F
                            nc.tensor.matmul(ps[:, h2, :],
                                             a_sb[:, 2 * g, 128 * f:128 * (f + 1)],
                                             sel[:, 0, 512 * e4:512 * (e4 + 1)],
                                             start=True, stop=False)
                            nc.tensor.matmul(ps[:, h2, :],
                                             a_sb[:, 2 * g + 1, 128 * f:128 * (f + 1)],
                                             sel[:, 1, 512 * e4:512 * (e4 + 1)],
                                             start=False, stop=pre)
                            if not pre:
                                nc.tensor.matmul(ps[:, h2, :],
                                                 wmc[k][0:16, 128 * f:128 * (f + 1)],
                                                 esg[0:16, 512 * e4:512 * (e4 + 1)],
                                                 start=False, stop=True)
                        dst = csbf[:, 1024 * i2:1024 * (i2 + 1)]
                        psf = ps[:].rearrange("p h n -> p (h n)")
                        if pre:
                            nc.vector.tensor_tensor(dst, dst, psf, AL.add)
                        else:
                            nc.scalar.activation(dst, psf, AF.Copy, bias=0.0)
                        if fillers:
                            fillers.pop(0)()
                    csbs[g] = csb

                def stage_B1(g):
                    p, gg = g // 2, g % 2
                    if gg == 0:
                        pairs[p] = (aggp.tile([128, F, 2, 256], BF, tag="pme", name="pme"),
                                    aggp.tile([128, F, 2, 256], BF, tag="pmx", name="pmx"),
                                    aggp.tile([128, F, 2, 256], BF, tag="pst", name="pst"),
                                    sml.tile([128, 2, F, 256], BF, tag="pm2", name="pm2"))
                    pme, pmx, pst, pm2 = pairs[p]
                    csb = csbs[g]
                    # layout [p, e(4), f, 512]: edge j = 2e + (i // 256), node n = i % 256
                    tscr = tpool.tile([128, 2, F, 512], BF, tag="tscr")
                    nc.vector.tensor_tensor(tscr[:], csb[:, 0:2, :, :],
                                            csb[:, 2:4, :, :], AL.max)
                    nc.vector.tensor_tensor(tscr[:, 0, :, :], tscr[:, 0, :, :],
                                            tscr[:, 1, :, :], AL.max)
                    nc.vector.tensor_tensor(pmx[:, :, gg, :], tscr[:, 0, :, 0:256],
                                            tscr[:, 0, :, 256:512], AL.max)
                    nc.vector.tensor_tensor(tscr[:], csb[:, 0:2, :, :],
                                            csb[:, 2:4, :, :], AL.add)
                    nc.vector.tensor_tensor(tscr[:, 0, :, :], tscr[:, 0, :, :],
                                            tscr[:, 1, :, :], AL.add)
                    ssum = qpool.tile([128, F, 256], BF, tag="ssum")
                    nc.vector.tensor_tensor(ssum[:], tscr[:, 0, :, 0:256],
                                            tscr[:, 0, :, 256:512], AL.add)
                    nc.vector.tensor_scalar(pme[:, :, gg, :], ssum[:], 0.125, None, AL.mult)
                    nc.vector.tensor_tensor(pm2[:, gg, :, :], pme[:, :, gg, :],
                                            pme[:, :, gg, :], AL.mult)

                def sq_fillers(g):
                    # square csb(g) in place, chunked by e4: two chunks emitted
                    # on the scalar queue between evictions (fillers), two on DVE
                    csb = csbs[g]
                    return [lambda e4=e4: nc.scalar.activation(
                        csb[:, e4, :, :], csb[:, e4, :, :], AF.Square) for e4 in (0, 1)]

                def stage_B2(g):
                    p, gg = g // 2, g % 2
                    pme, pmx, pst, pm2 = pairs[p]
                    csb = csbs.pop(g)
                    for e4 in (2, 3):
                        nc.vector.tensor_tensor(csb[:, e4, :, :], csb[:, e4, :, :],
                                                csb[:, e4, :, :], AL.mult)
                    tscr = tpool.tile([128, 2, F, 512], BF, tag="tscr")
                    nc.vector.tensor_tensor(tscr[:], csb[:, 0:2, :, :],
                                            csb[:, 2:4, :, :], AL.add)
                    nc.vector.tensor_tensor(tscr[:, 0, :, :], tscr[:, 0, :, :],
                                            tscr[:, 1, :, :], AL.add)
                    qsum = qpool.tile([128, F, 256], F32, tag="qsum")
                    nc.vector.tensor_tensor(qsum[:], tscr[:, 0, :, 0:256],
                                            tscr[:, 0, :, 256:512], AL.add)
                    # var = relu(qsum/8 - mean^2), std = sqrt(var + 1e-30)
                    nc.vector.tensor_scalar(qsum[:], qsum[:], 0.125, None, AL.mult)
                    nc.vector.tensor_tensor(qsum[:], qsum[:], pm2[:, gg, :, :], AL.subtract)
                    nc.vector.tensor_scalar(qsum[:], qsum[:], 0.0, None, AL.max)
                    nc.scalar.activation(pst[:, :, gg, :], qsum[:], AF.Sqrt, bias=c30[:, 0:1])

                def stage_U(p):
                    pme, pmx, pst, pm2 = pairs.pop(p)
                    xs = [None, pme, pmx, pst]
                    for mo in range(Fo):
                        mi = msz[mo]
                        ps = psU.tile([128, 512], F32, tag="psU")
                        nmm = 4 * F
                        i = 0
                        for sect in range(4):
                            for f in range(F):
                                if sect == 0:
                                    rhs = h[0:csz[f], f, 512 * p:512 * (p + 1)]
                                else:
                                    rhs = xs[sect][0:csz[f], f, :, :]
                                nc.tensor.matmul(
                                    ps[0:mi, :],
                                    wu_k[0:csz[f], sect * F + f, 128 * mo:128 * mo + mi],
                                    rhs, start=(i == 0), stop=(i == nmm - 1))
                                i += 1
                        nc.scalar.activation(uT[0:mi, mo, 512 * p:512 * (p + 1)], ps[0:mi, :],
                                             AF.Copy, bias=0.0,
                                             accum_out=uaccS[0:mi, mo, p:p + 1])
                        usq = sml.tile([128, 512], BF, tag="usq")
                        nc.scalar.activation(usq[0:mi, :], uT[0:mi, mo, 512 * p:512 * (p + 1)],
                                             AF.Square, accum_out=uaccQ[0:mi, mo, p:p + 1])

                # ---- layer-0 pair-granular stages (F=1: ops are tiny, so the
                # per-op latency dominates; processing both graphs of a pair in
                # each op halves the chain length) --------------------------------
                def stage_A1(p, fillers):
                    csb = cpool.tile([128, 2, 4, 512], BF, tag="csb", name="csb1")
                    csbf = csb[:].rearrange("p g a n -> p (g a n)")
                    for gg in range(2):
                        g = 2 * p + gg
                        sel = selp.tile([128, 2, 2048], BF, tag="sel")
                        nc.sync.dma_start(sel[:].rearrange("p b n -> p (b n)"),
                                          p_sel[:, 4096 * g:4096 * (g + 1)])
                        esg = espool.tile([16, 2048], BF, tag="esg")
                        nc.sync.dma_start(esg[:], p_esT[0:16, 2048 * g:2048 * (g + 1)])
                        for i2 in range(2):
                            ps = psC.tile([128, 2, 512], F32, tag="psC")
                            for h2 in range(2):
                                e4 = 2 * i2 + h2
                                nc.tensor.matmul(ps[:, h2, :],
                                                 a_sb[:, 2 * g, 0:128],
                                                 sel[:, 0, 512 * e4:512 * (e4 + 1)],
                                                 start=True, stop=False)
                                nc.tensor.matmul(ps[:, h2, :],
                                                 a_sb[:, 2 * g + 1, 0:128],
                                                 sel[:, 1, 512 * e4:512 * (e4 + 1)],
                                                 start=False, stop=False)
                                nc.tensor.matmul(ps[:, h2, :],
                                                 wmc[k][0:16, 0:128],
                                                 esg[0:16, 512 * e4:512 * (e4 + 1)],
                                                 start=False, stop=True)
                            dst = csbf[:, (4 * gg + 2 * i2) * 512:(4 * gg + 2 * i2 + 2) * 512]
                            nc.scalar.activation(dst, ps[:].rearrange("p h n -> p (h n)"),
                                                 AF.Copy, bias=0.0)
                            if fillers:
                                fillers.pop(0)()
                    csbs[p] = csb

                def sq1_fillers(p):
                    csb = csbs[p]
                    return [lambda e4=e4: nc.scalar.activation(
                        csb[:, :, e4, :], csb[:, :, e4, :], AF.Square) for e4 in (0, 1)]

                def stage_B1_1(p):
                    pairs[p] = (aggp.tile([128, F, 2, 256], BF, tag="pme", name="pme"),
                                aggp.tile([128, F, 2, 256], BF, tag="pmx", name="pmx"),
                                aggp.tile([128, F, 2, 256], BF, tag="pst", name="pst"),
                                sml.tile([128, 2, F, 256], BF, tag="pm2", name="pm2"))
                    pme, pmx, pst, pm2 = pairs[p]
                    csb = csbs[p]
                    tscr = tpool.tile([128, 2, 2, 512], BF, tag="tscr", name="tscr1")
                    nc.vector.tensor_tensor(tscr[:], csb[:, :, 0:2, :],
                                            csb[:, :, 2:4, :], AL.max)
                    nc.vector.tensor_tensor(tscr[:, :, 0, :], tscr[:, :, 0, :],
                                            tscr[:, :, 1, :], AL.max)
                    nc.vector.tensor_tensor(pmx[:, 0, :, :], tscr[:, :, 0, 0:256],
                                            tscr[:, :, 0, 256:512], AL.max)
                    nc.vector.tensor_tensor(tscr[:], csb[:, :, 0:2, :],
                                            csb[:, :, 2:4, :], AL.add)
                    nc.vector.tensor_tensor(tscr[:, :, 0, :], tscr[:, :, 0, :],
                                            tscr[:, :, 1, :], AL.add)
                    ssum = qpool.tile([128, 2, 256], BF, tag="ssum", name="ssum1")
                    nc.vector.tensor_tensor(ssum[:], tscr[:, :, 0, 0:256],
                                            tscr[:, :, 0, 256:512], AL.add)
                    nc.vector.tensor_scalar(pme[:, 0, :, :], ssum[:], 0.125, None, AL.mult)
                    nc.vector.tensor_tensor(pm2[:, :, 0, :], pme[:, 0, :, :],
                                            pme[:, 0, :, :], AL.mult)

                def stage_B2_1(p):
                    pme, pmx, pst, pm2 = pairs[p]
                    csb = csbs.pop(p)
                    nc.vector.tensor_tensor(csb[:, :, 2:4, :], csb[:, :, 2:4, :],
                                            csb[:, :, 2:4, :], AL.mult)
                    tscr = tpool.tile([128, 2, 2, 512], BF, tag="tscr", name="tscr1")
                    nc.vector.tensor_tensor(tscr[:], csb[:, :, 0:2, :],
                                            csb[:, :, 2:4, :], AL.add)
                    nc.vector.tensor_tensor(tscr[:, :, 0, :], tscr[:, :, 0, :],
                                            tscr[:, :, 1, :], AL.add)
                    qsum = qpool.tile([128, 2, 256], F32, tag="qsum", name="qsum1")
                    nc.vector.tensor_tensor(qsum[:], tscr[:, :, 0, 0:256],
                                            tscr[:, :, 0, 256:512], AL.add)
                    nc.vector.tensor_scalar(qsum[:], qsum[:], 0.125, None, AL.mult)
                    nc.vector.tensor_tensor(qsum[:], qsum[:], pm2[:, :, 0, :], AL.subtract)
                    nc.vector.tensor_scalar(qsum[:], qsum[:], 0.0, None, AL.max)
                    nc.scalar.activation(pst[:, 0, :, :], qsum[:], AF.Sqrt, bias=c30[:, 0:1])

                if k == 0:
                    for it in range(4 + 2):
                        fillers = sq1_fillers(it - 2) if 2 <= it <= 3 + 2 else []
                        if it < 4:
                            stage_A1(it, fillers)
                        for fop in fillers:
                            fop()
                        if 1 <= it <= 4:
                            stage_B1_1(it - 1)
                        if it >= 2:
                            stage_B2_1(it - 2)
                            stage_U(it - 2)
                else:
                    for it in range(G + 3):
                        fillers = sq_fillers(it - 2) if 2 <= it <= G + 1 else []
                        if it < G:
                            stage_A(it, fillers)
                        for fop in fillers:
                            fop()
                        if 1 <= it <= G:
                            stage_B1(it - 1)
                        if 2 <= it <= G + 1:
                            stage_B2(it - 2)
                        if it >= 3 and (it - 3) % 2 == 1:
                            stage_U((it - 3) // 2)

                # ---- prefetch next layer's C = es @ Wmc into the AR window ----
                cpre = []
                if k < 2:
                    F1 = NF[k + 1]
                    for gp in range(PREF):
                        esg = espool.tile([16, 2048], BF, tag="esg")
                        nc.sync.dma_start(esg[:], p_esT[0:16, 2048 * gp:2048 * (gp + 1)])
                        cp = cpool.tile([128, 4, F1, 512], BF, tag="csb")
                        cpf = cp[:].rearrange("p a f n -> p (a f n)")
                        for i2 in range(2 * F1):
                            ps = psC.tile([128, 2, 512], F32, tag="psC")
                            for h2 in range(2):
                                flat = 2 * i2 + h2
                                e4, f = flat // F1, flat % F1
                                nc.tensor.matmul(ps[:, h2, :],
                                                 wmc[k + 1][0:16, 128 * f:128 * (f + 1)],
                                                 esg[0:16, 512 * e4:512 * (e4 + 1)],
                                                 start=True, stop=True)
                            dst = cpf[:, 1024 * i2:1024 * (i2 + 1)]
                            psf = ps[:].rearrange("p h n -> p (h n)")
                            if i2 % 2 == 0:
                                nc.scalar.activation(dst, psf, AF.Copy, bias=0.0)
                            else:
                                nc.vector.tensor_copy(dst, psf)
                        cpre.append(cp)

                # ---- BN stats all-reduce, fold into mixing ----
                # chunk-reduce on the scalar queue (fires right after the last
                # U evict; the vector queue still has a tree backlog here)
                ccs = stat.tile([128, 6], F32, tag="ccs")
                racc = sml.tile([128, 4], BF, tag="usq", name="racc")
                for mo in range(Fo):
                    nc.scalar.activation(racc[:, :], uaccS[:, mo, :], AF.Copy, bias=0.0,
                                         accum_out=ccs[:, mo:mo + 1])
                    nc.scalar.activation(racc[:, :], uaccQ[:, mo, :], AF.Copy, bias=0.0,
                                         accum_out=ccs[:, Fo + mo:Fo + mo + 1])
                nc.sync.dma_start(cc_in[k][:], ccs[:, 0:2 * Fo])
                nc.gpsimd.collective_compute(
                    "AllReduce", AL.add, replica_groups=_rg,
                    ins=[cc_in[k].opt()], outs=[cc_out[k].opt()])
                ccr = stat.tile([128, 6], F32, tag="ccr")
                nc.sync.dma_start(ccr[:, 0:2 * Fo], cc_out[k][:])
                mu = stat.tile([128, 3], F32, tag="mu")
                sc = stat.tile([128, 3], F32, tag="sc")
                mu2 = stat.tile([128, 3], F32, tag="mu2")
                nc.scalar.activation(mu[:, 0:Fo], ccr[:, 0:Fo], AF.Copy, bias=0.0, scale=1.0 / N)
                nc.scalar.activation(sc[:, 0:Fo], ccr[:, Fo:2 * Fo], AF.Copy, bias=0.0, scale=1.0 / N)
                nc.vector.tensor_tensor(mu2[:, 0:Fo], mu[:, 0:Fo], mu[:, 0:Fo], AL.mult)
                nc.vector.tensor_tensor(sc[:, 0:Fo], sc[:, 0:Fo], mu2[:, 0:Fo], AL.subtract)
                nc.scalar.activation(sc[:, 0:Fo], sc[:, 0:Fo], AF.Sqrt, bias=c5[:, 0:1])
                nc.vector.reciprocal(sc[:, 0:Fo], sc[:, 0:Fo])
                nc.vector.tensor_tensor(sc[:, 0:Fo], sc[:, 0:Fo], gam[k][:, 0:Fo], AL.mult)
                for mo in range(Fo):
                    mi = msz[mo]
                    nc.vector.tensor_scalar(uT[0:mi, mo, :], uT[0:mi, mo, :],
                                            mu[0:mi, mo:mo + 1], None, AL.subtract)
                    nc.vector.tensor_scalar(wxs[:, mo, 0:cout], wx[k][:, mo, 0:cout],
                                            sc[:, mo:mo + 1], None, AL.mult)
                if k == 2:
                    nc.vector.memset(uT[64:65, 1, :], 1.0)
                # mixing matmul (+ BN shift via bias / ones-row), relu(leaky) = relu
                if k < 2:
                    hn = hT[k + 1]
                    for mo in range(Fo):
                        for n4 in range(4):
                            ps = psW.tile([128, 512], F32, tag="psW")
                            for mk in range(Fo):
                                nc.tensor.matmul(ps[:, :],
                                                 wxs[0:msz[mk], mk, 128 * mo:128 * (mo + 1)],
                                                 uT[0:msz[mk], mk, 512 * n4:512 * (n4 + 1)],
                                                 start=(mk == 0), stop=(mk == Fo - 1))
                            nc.scalar.activation(hn[:, mo, 512 * n4:512 * (n4 + 1)], ps[:, :],
                                                 AF.Relu, bias=bh[k][:, mo:mo + 1])
                else:
                    nmx = stat.tile([128, 16], BF, tag="nmx")
                    for t in range(16):
                        ps = psW.tile([128, 512], F32, tag="psW")
                        nc.tensor.matmul(ps[:, 0:192], uT[0:128, 0, 128 * t:128 * (t + 1)],
                                         wxs[0:128, 0, 0:192], start=True, stop=False)
                        nc.tensor.matmul(ps[:, 0:192], uT[0:65, 1, 128 * t:128 * (t + 1)],
                                         wxs[0:65, 1, 0:192], start=False, stop=True)
                        nc.scalar.activation(h3[:, t, :], ps[:, 0:192], AF.Lrelu, alpha=0.01)
                        nc.vector.tensor_reduce(nmx[:, t:t + 1], h3[:, t:t + 1, :], AX.X, AL.max)

            # ---- head --------------------------------------------------------
            ps3 = psW.tile([128, 512], F32, tag="psW")
            nc.tensor.matmul(ps3[0:64, 0:8], w3[:, 0, :], nmx[:, 0::2], start=True, stop=False)
            nc.tensor.matmul(ps3[0:64, 0:8], w3[:, 1, :], nmx[:, 1::2], start=False, stop=True)
            r3 = stat.tile([64, 8], BF, tag="r3")
            nc.scalar.activation(r3[:], ps3[0:64, 0:8], AF.Relu, bias=b3[:, 0:1])
            gn = stat.tile([128, 16], F32, tag="gn")
            for half in range(2):
                ps4 = psW.tile([128, 512], F32, tag="psW")
                nc.tensor.matmul(ps4[:, 0:8], w4[0:64, 128 * half:128 * (half + 1)], r3[:],
                                 start=True, stop=True)
                nc.scalar.activation(gn[:, half::2], ps4[:, 0:8], AF.Sigmoid,
                                     bias=b4[:, half:half + 1])
            mask = cpool.tile([128, 16, 192], BF, tag="csb", name="maskt")
            nc.scalar.dma_start(mask[:], p_mask[:])
            feat = cpool.tile([128, 16, 192], F32, tag="csb")
            for c in range(16):
                nc.vector.tensor_scalar(feat[:, c, :], h3[:, c, :], gn[:, c:c + 1], None, AL.mult)
            # softmax shifted by the unmasked max (shift-invariant); mask after exp
            gmax = stat.tile([128, 8], F32, tag="gmax")
            gmaxr = stat.tile([128, 8], F32, tag="gmaxr")
            nc.vector.tensor_reduce(gmax[:], feat[:].rearrange("p (g x) t -> p g (x t)", g=8), AX.X, AL.max)
            nc.gpsimd.partition_all_reduce(gmaxr[:], gmax[:], 128, bass_isa.ReduceOp.max)
            nc.vector.tensor_scalar(gmaxr[:], gmaxr[:], -1.0, None, AL.mult)
            for g in range(8):
                nc.scalar.activation(feat[:, 2 * g:2 * (g + 1), :], feat[:, 2 * g:2 * (g + 1), :],
                                     AF.Exp, bias=gmaxr[:, g:g + 1])
            nc.vector.tensor_tensor(feat[:], feat[:], mask[:], AL.mult)
            gsum = stat.tile([128, 8], F32, tag="gsum")
            gsumr = stat.tile([128, 8], F32, tag="gsumr")
            nc.vector.tensor_reduce(gsum[:], feat[:].rearrange("p (g x) t -> p g (x t)", g=8), AX.X, AL.add)
            nc.gpsimd.partition_all_reduce(gsumr[:], gsum[:], 128, bass_isa.ReduceOp.add)
            nc.vector.reciprocal(gsumr[:], gsumr[:])
            osb = cpool.tile([128, 16, 192], F32, tag="csb")
            for g in range(8):
                nc.vector.tensor_scalar(osb[:, 2 * g:2 * (g + 1), :], feat[:, 2 * g:2 * (g + 1), :],
                                        gsumr[:, g:g + 1], None, AL.mult)
            nc.sync.dma_start(p_out[:], osb[:].rearrange("p c t -> p (c t)"))

    nc.compile()
    return nc


# ---------------------------------------------------------------------------
# host prep + launch
# ---------------------------------------------------------------------------

def prepare_in_maps(inputs):
    src = np.asarray(inputs["src"], np.int64)
    dst = np.asarray(inputs["dst"], np.int64)
    assert np.array_equal(dst, np.repeat(np.arange(N, dtype=np.int64), DEG)), "dst structure"
    assert np.array_equal(src // NN, dst // NN), "edges must be graph-local"

    ns = _f32(inputs["ns"]); es = _f32(inputs["es"]); dm = _f32(inputs["dm"])
    mask_fv = _f32(inputs["mask_fv"])

    Wm = [_f32(inputs[f"Wm{k + 1}"]) for k in range(3)]
    Wu = [_f32(inputs[f"Wu{k + 1}"]) for k in range(3)]
    Wx = [_f32(inputs[f"Wx{k + 1}"]) for k in range(3)]
    bx = [_f32(inputs[f"bx{k + 1}"]) for k in range(3)]
    bng = [_f32(inputs[f"bng{k + 1}"]) for k in range(3)]
    bnb = [_f32(inputs[f"bnb{k + 1}"]) for k in range(3)]

    wma_u, wmc_u, wu_u, wx_u, gam_u, bh_u = [], [], [], [], [], []
    for k in range(3):
        cin, cout, Fk, cinp, Fo = CIN[k], COUT[k], NF[k], CINP[k], NFO[k]
        Wma, Wmb, Wmce = Wm[k][:cin], Wm[k][cin:2 * cin], Wm[k][2 * cin:]
        Wmean = Wu[k][cin:2 * cin] + 8.0 * Wu[k][3 * cin:4 * cin]
        Wmax = Wu[k][2 * cin:3 * cin]
        Wstd = Wu[k][4 * cin:]
        Wh = Wu[k][:cin] + Wmb @ (Wmean + Wmax)
        a = np.zeros((128, Fk, cinp), np.float32)
        for ki in range(Fk):
            a[0:CSZ[k][ki], ki, :cin] = Wma[128 * ki:128 * ki + CSZ[k][ki]]
        wma_u.append(_bf(a.reshape(128, -1)))
        c = np.zeros((16, cinp), np.float32)
        c[:, :cin] = Wmce
        wmc_u.append(_bf(c))
        u = np.zeros((128, 4 * Fk, cout), np.float32)
        for si, Wsec in enumerate([Wh, Wmean, Wmax, Wstd]):
            for f in range(Fk):
                u[0:CSZ[k][f], si * Fk + f, :] = Wsec[128 * f:128 * f + CSZ[k][f]]
        wu_u.append(_bf(u.reshape(128, -1)))
        if k < 2:
            x = np.zeros((128, Fo, cout), np.float32)
            gcol = np.zeros((128, Fo), np.float32)
            bcol = np.zeros((128, Fo), np.float32)
            bhv = bnb[k] @ Wx[k] + bx[k]
            for mk in range(Fo):
                m = MSZ[k][mk]
                x[0:m, mk, :] = Wx[k][128 * mk:128 * mk + m]
                gcol[0:m, mk] = bng[k][128 * mk:128 * mk + m]
                bcol[0:m, mk] = bhv[128 * mk:128 * mk + m]
            wx_u.append(_bf(x.reshape(128, -1)))
            gam_u.append(_f32(gcol))
            bh_u.append(_f32(bcol))
        else:
            x = np.zeros((128, 2, cout), np.float32)
            x[0:128, 0, :] = Wx[k][0:128]
            x[0:64, 1, :] = Wx[k][128:192]
            x[64, 1, :] = bnb[k] @ Wx[k] + bx[k]       # bias row (pairs with u ones-row)
            wx_u.append(_bf(x.reshape(128, -1)))
            gcol = np.zeros((128, 2), np.float32)
            gcol[0:128, 0] = bng[k][0:128]
            gcol[0:64, 1] = bng[k][128:192]
            gcol[64, 1] = np.sqrt(np.float32(1e-5))    # scale row becomes exactly 1.0
            gam_u.append(_f32(gcol))

    W12 = _f32(inputs["W1"]) @ _f32(inputs["W2"])
    b12v = _f32(inputs["b1"]) @ _f32(inputs["W2"]) + _f32(inputs["b2"])
    w12_u = _bf(W12.reshape(2, 128, 32).transpose(1, 0, 2).reshape(128, -1))
    w3_u = _bf(_f32(inputs["W3"]).reshape(2, 128, 64).transpose(1, 0, 2).reshape(128, -1))
    w4_u = _bf(inputs["W4"])
    b4_u = _f32(np.asarray(inputs["b4"]).reshape(2, 128).T)

    shared = {
        **{f"wma{k}": wma_u[k] for k in range(3)},
        **{f"wmc{k}": wmc_u[k] for k in range(3)},
        **{f"wu{k}": wu_u[k] for k in range(3)},
        **{f"wx{k}": wx_u[k] for k in range(3)},
        **{f"gam{k}": gam_u[k] for k in range(3)},
        **{f"bh{k}": bh_u[k] for k in range(2)},
        "w12": w12_u, "b12": _f32(b12v.reshape(32, 1)),
        "w3": w3_u, "b3": _f32(np.asarray(inputs["b3"]).reshape(64, 1)),
        "w4": w4_u, "b4": b4_u,
    }

    in_maps = []
    ecols = np.arange(2048)
    for c in range(NCORES):
        n0 = NC * c
        gg, jj, nn2 = np.meshgrid(np.arange(G), np.arange(DEG), np.arange(NN), indexing="ij")
        perm = (8 * (n0 + 256 * gg + nn2) + jj).reshape(-1)
        esl = es[perm]
        # per-graph one-hot src selectors: sel[row, g, nb, j*256+n] = 1
        # iff src of edge (g, j, n) is node nb*128 + row of graph g
        srcg = (src[perm] - n0).reshape(G, DEG * NN).astype(np.int64)
        srcg -= 256 * (np.arange(G)[:, None])            # graph-local 0..255
        sel = np.zeros((128, G, 2, 2048), ml_dtypes.bfloat16)
        gidx = np.repeat(np.arange(G), 2048)
        sel[(srcg % 128).reshape(-1), gidx, (srcg // 128).reshape(-1), np.tile(ecols, G)] = 1.0
        in_maps.append({
            "nsT": _bf(ns[n0:n0 + NC].T),
            "dmT": _bf(dm[n0:n0 + NC].T.reshape(2, 128, 2048).transpose(1, 0, 2).reshape(128, -1)),
            "esT": _bf(esl.T),
            "sel": np.ascontiguousarray(sel.reshape(128, -1)),
            "mask": _bf(mask_fv[n0:n0 + NC].reshape(16, 128, 192).transpose(1, 0, 2)
                        .reshape(128, -1)),
            **shared,
        })

    return in_maps


def collect_out(res):
    out = np.zeros((B, NN * TP), np.float32)
    for c in range(NCORES):
        oc = res.results[c]["out"].reshape(128, 16, 192).transpose(1, 0, 2).reshape(NC, TP)
        out[G * c:G * (c + 1)] = oc.reshape(G, NN * TP)
    return out


def kernel(**inputs):
    in_maps = prepare_in_maps(inputs)
    nc = _BUILT.get("nc")
    if nc is None:
        nc = build_nc()
        _BUILT["nc"] = nc
    res = run_bass_kernel_spmd(nc, in_maps, list(range(NCORES)))
    _BUILT["last_results"] = res
    return collect_out(res)
